# revision 30
# baseline (speedup 1.0000x reference)
"""Trainium2 Bass kernel for 16-head MHA (B=2, S=2048, D=1024), fp32 I/O.

v2: score matmuls head-interleaved (h0/h1 K-slices at partition base 0/64
-> distinct PE row groups -> concurrent streaming), wo drain on ACT as fp16
with 1MB SWDGE out-DMAs (sync ring reserved for the input stream), 50/50
ACT/DVE exp checkerboard, vpt copies on gpsimd, 8-deep input prefetch.

Sharding: tensor-parallel by heads across 8 NeuronCores. Core c owns heads
2c, 2c+1 (a 128-wide slice of the QKV projection output and of Wo's input
dim). Each core computes its head group's full attention plus a partial
output projection; the host sums the 8 partials.

Per-core dataflow (feature-major so the PE contraction dim is always the
SBUF partition dim; the host pre-transposes q/k/v and weights and casts
to fp16 -- same PE speed as bf16, 8x finer mantissa):

  projections: TT = W_c @ x.T streamed as half-batch k-tiles
    [128, 2048] (4 KB DMA lines, all on the sync queue), accumulated in
    one [128, 2048] PSUM tile, emitted interleaved with the first two
    attention windows so the 24 MB input DMA overlaps scores/exp.
  V+ tiles ([128j, 16jc, 64+1] per (head, batch), the 65th column ones
    for the softmax denominator): one XBAR DMA-transpose per (head,
    batch) into a contiguous staging tile + a strided DVE copy.
  window pipeline (4 windows of 1024 query cols, 16 j-chunk positions,
  three stages overlap):
    scores(L):  S.T [128j, 1024i] = KT.T @ QT per head -> exp on ACT
                (activation Exp) or DVE via the Schraudolph identity
                fp16(exp(s/8)) == bitcast<u16>(s*184.66*SCALE + 15315)
                (f32->u16 saturates negatives to 0 == exp underflow;
                ~3% max rel err on its share of weights). Checkerboard
                (2p+h)%3 keeps both engines under the PE cadence.
    av(L-1):    O+ [65, 1024] += V+.T @ E, start-delayed 2 positions
                (8 positions in loop 1, which waits out the input DMA).
    normalize(L-2): reciprocal_approx_fast on the PSUM denominator row,
                replicate via gpsimd partition_broadcast, OC = O+ * rep.
    wo(L-2):    out rows = OC.T @ WoT_c (fp32r), positions 4..11, DVE
                PSUM->SBUF copy, out DMA on sync.
"""

import sys

sys.path.insert(0, "/opt/trn_rl_repo")

import numpy as np

import concourse.bacc as bacc
import concourse.mybir as mybir
import concourse.tile as tile
from concourse.bass_utils import run_bass_kernel_spmd

F32 = mybir.dt.float32
R = mybir.dt.float32r
F16 = mybir.dt.float16
U16 = mybir.dt.uint16
EXP = mybir.ActivationFunctionType.Exp
COPY = mybir.ActivationFunctionType.Copy
MULT = mybir.AluOpType.mult
ADD = mybir.AluOpType.add

D = 1024
BATCH = 2
SEQ = 2048
M = BATCH * SEQ  # 4096 token rows
HEADS_PER_CORE = 2
DK = 64
HG = HEADS_PER_CORE * DK  # 128-wide head-group slice per core
N_CORES = 8
KT_TILES = D // 128  # 8 contraction tiles for the projections
JC = SEQ // 128  # 16 j-chunks per batch
N_WIN = 4  # (b, ih) windows of 1024 query columns
SCALE = 1.0 / np.sqrt(DK)

# Schraudolph exp in fp16 bits, C=45 tuned offline for min max-rel-err (3.0%)
SCH_A = float(np.log2(np.e) * 1024.0) * SCALE
SCH_B = 15360.0 - 45.0


def build_bass():
    nc = bacc.Bacc(None)

    qT = nc.dram_tensor("qT", [D, M], F16, kind="ExternalInput")
    kT = nc.dram_tensor("kT", [D, M], F16, kind="ExternalInput")
    vT = nc.dram_tensor("vT", [D, M], F16, kind="ExternalInput")
    # host pre-arranges projection weights as [p, ko, n] so the load is one
    # contiguous 2 KB/partition DMA instead of 256 B strided descriptors
    wqT = nc.dram_tensor("wqT", [128, KT_TILES, HG], F16, kind="ExternalInput")
    wkT = nc.dram_tensor("wkT", [128, KT_TILES, HG], F16, kind="ExternalInput")
    wvT = nc.dram_tensor("wvT", [128, KT_TILES, HG], F16, kind="ExternalInput")
    woT = nc.dram_tensor("woT", [HG, D], R, kind="ExternalInput")
    out = nc.dram_tensor("out", [M, D], F16, kind="ExternalOutput")
    # [quad, p, c, d] view: quad q covers rows q*512..q*512+512 as 4 chunks
    out_q = out.rearrange("(q c p) d -> q p c d", p=128, c=4)

    with tile.TileContext(nc) as tc:
        with (
            tc.tile_pool(name="consts", bufs=1) as cst,
            tc.tile_pool(name="acts", bufs=1) as acts,
            tc.tile_pool(name="vp", bufs=1) as vp_pool,
            tc.tile_pool(name="ocpool", bufs=2) as ocpool,
            tc.tile_pool(name="outpool", bufs=2) as outpool,
            tc.tile_pool(name="small", bufs=1) as small,
            tc.tile_pool(name="epool", bufs=46) as epool,
            tc.tile_pool(name="psb", bufs=2, space="PSUM") as psb,
        ):
            ones_f = cst.tile([128, 1], F32)
            nc.gpsimd.memset(ones_f[:], 1.0)
            # warm the ACT exp table while DMA streams inputs
            scratch = cst.tile([1, 64], F32)
            nc.scalar.activation(
                scratch[:], ones_f[0:1, 0:1].to_broadcast([1, 64]), EXP
            )

            wo_sb = acts.tile([HG, D], R)
            nc.sync.dma_start(wo_sb[:], woT[:])

            QT = acts.tile([HG, M], F16)
            KT = acts.tile([HG, M], F16)
            VT = acts.tile([HG, M], F16)

            vp_tiles = {}
            windows = [(b, ih) for b in range(BATCH) for ih in range(2)]

            def emit_scores(st, p):
                # h0/h1 matmuls interleaved: K slices live at partition base
                # 0 / 64, so consecutive MMs land in distinct PE row groups
                # and stream concurrently (2x scores throughput).
                b, ih = st["w"]
                i0 = b * SEQ + ih * 1024
                j0 = b * SEQ + p * 128
                ps = [
                    psb.tile([128, 1024], F32, tag="big", name=f"ps{h}")
                    for h in range(HEADS_PER_CORE)
                ]
                for iw in range(2):
                    for h in range(HEADS_PER_CORE):
                        hs = slice(h * DK, (h + 1) * DK)
                        nc.tensor.matmul(
                            ps[h][:, iw * 512 : (iw + 1) * 512],
                            KT[hs, j0 : j0 + 128],
                            QT[hs, i0 + iw * 512 : i0 + (iw + 1) * 512],
                            start=True,
                            stop=True,
                        )
                for h in range(HEADS_PER_CORE):
                    e_t = epool.tile([128, 1024], F16, tag="e")
                    if (2 * p + h) % 16 < 9:
                        nc.vector.tensor_scalar(
                            e_t[:].bitcast(U16), ps[h][:], SCH_A, SCH_B, MULT, ADD
                        )
                    else:
                        nc.scalar.activation(e_t[:], ps[h][:], EXP, scale=SCALE)
                    st["e"][(h, p)] = e_t

            def emit_av(st, pso, chunks):
                b = st["w"][0]
                for jc in chunks:
                    if jc == 0:
                        st["po"] = {
                            h: pso.tile(
                                [DK + 1, 1024], F32, tag="po", name=f"po{h}"
                            )
                            for h in range(HEADS_PER_CORE)
                        }
                    po, e_tiles = st["po"], st["e"]
                    for h in range(HEADS_PER_CORE):
                        for iw in range(2):
                            nc.tensor.matmul(
                                po[h][:, iw * 512 : (iw + 1) * 512],
                                vp_tiles[(h, b)][:, jc, :],
                                e_tiles[(h, jc)][:, iw * 512 : (iw + 1) * 512],
                                start=(jc == 0),
                                stop=(jc == JC - 1),
                            )

            def emit_normalize(st):
                po = st["po"]
                oc = ocpool.tile([HG, 1024], R, tag="oc")
                # dn drains on ACT (parallel with DVE exps); one batched DVE
                # reciprocal covers both heads -- shortens the serialized
                # window-boundary chain that gates the next window's AV
                dn = small.tile([1, 2048], F32, tag="dn", name="dn")
                for h in range(HEADS_PER_CORE):
                    nc.scalar.activation(
                        dn[0:1, h * 1024 : (h + 1) * 1024],
                        po[h][DK : DK + 1, :],
                        COPY,
                    )
                rr = small.tile([1, 2048], F32, tag="rr", name="rr")
                nc.vector.reciprocal_approx_fast(rr[:], dn[:])
                for h in range(HEADS_PER_CORE):
                    hs = slice(h * DK, (h + 1) * DK)
                    rb = small.tile([64, 1024], F32, tag=f"rb{h}", name=f"rb{h}")
                    nc.gpsimd.partition_broadcast(
                        rb[:], rr[0:1, h * 1024 : (h + 1) * 1024]
                    )
                    nc.vector.tensor_tensor(oc[hs, :], po[h][0:DK, :], rb[:], MULT)
                st["oc"] = oc

            def emit_wo(st, ic):
                # out rows drain via ACT (fp16 cast) into a 4-chunk staging
                # tile; one 1 MB SWDGE DMA per 512-row quad keeps the sync
                # ring free for the input stream.
                b, ih = st["w"]
                oc = st["oc"]
                wo_ps = psb.tile([128, 1024], F32, tag="big")
                for oh in range(2):
                    nc.tensor.matmul(
                        wo_ps[:, oh * 512 : (oh + 1) * 512],
                        oc[:, ic * 128 : (ic + 1) * 128],
                        wo_sb[:, oh * 512 : (oh + 1) * 512],
                        start=True,
                        stop=True,
                    )
                seg = ic % 4
                if seg == 0:
                    st["ostg"] = outpool.tile(
                        [128, 4, 1024], F16, tag="os", name="ostg"
                    )
                nc.scalar.activation(st["ostg"][:, seg, :], wo_ps[:], COPY)
                if seg == 3:
                    q = (b * 2 + ih) * 2 + ic // 4
                    nc.gpsimd.dma_start(out_q[q], st["ostg"][:])

            # av starts at p=4 (covers the window-boundary normalize chain),
            # catches up with 2-chunk positions at the window end
            AV_CHUNKS = {p: [] for p in range(JC)}
            for p in range(4, 12):
                AV_CHUNKS[p].append(p - 4)
            for p in range(12, 16):
                AV_CHUNKS[p] = [2 * (p - 12) + 8, 2 * (p - 12) + 9]
            AV1_CHUNKS = {p: [2 * (p - 8), 2 * (p - 8) + 1] for p in range(8, 16)}
            WO_POS = {p: p - 4 for p in range(4, 12)}

            with (
                tc.tile_pool(name="wpool", bufs=1) as wpool,
                tc.tile_pool(name="stage", bufs=5) as stage,
                tc.tile_pool(name="pp", bufs=1, space="PSUM") as pp,
            ):
                wq_sb = wpool.tile([128, KT_TILES, HG], F16)
                wk_sb = wpool.tile([128, KT_TILES, HG], F16)
                wv_sb = wpool.tile([128, KT_TILES, HG], F16)
                for w_sb, w_dram in ((wk_sb, wkT), (wq_sb, wqT), (wv_sb, wvT)):
                    nc.sync.dma_start(w_sb[:], w_dram[:])

                def emit_half_kstep(TT, w_sb, x_dram, b, k, pq_box):
                    base = b * 2048
                    if k == 0:
                        pq_box[0] = pp.tile(
                            [128, 2048], F32, tag="pq", name="pq"
                        )
                    pq = pq_box[0]
                    xst = stage.tile([128, 2048], F16, tag="xst", name="xst")
                    # V tiles ride the scalar HWDGE ring so the two input
                    # streams run in parallel toward the per-NC HBM cap
                    dq = nc.scalar if x_dram is vT else nc.sync
                    dq.dma_start(
                        xst[:], x_dram[k * 128 : (k + 1) * 128, base : base + 2048]
                    )
                    for nh in range(4):
                        nc.tensor.matmul(
                            pq[:, nh * 512 : (nh + 1) * 512],
                            w_sb[:, k, :],
                            xst[:, nh * 512 : (nh + 1) * 512],
                            start=(k == 0),
                            stop=(k == KT_TILES - 1),
                        )
                    if k == KT_TILES - 1:
                        nc.vector.tensor_copy(TT[:, base : base + 2048], pq[:])

                def emit_vplus(b):
                    for h in range(HEADS_PER_CORE):
                        hs = slice(h * DK, (h + 1) * DK)
                        tmp = vp_pool.tile(
                            [128, JC, DK], F16, tag="vtmp", bufs=2, name="vtmp"
                        )
                        nc.scalar.dma_start(
                            tmp[:], VT[hs, b * 2048 : (b + 1) * 2048],
                            transpose=True,
                        )
                        vpt = vp_pool.tile(
                            [128, JC, DK + 1], F16, tag=f"vp_{h}_{b}",
                            name=f"vp_{h}_{b}",
                        )
                        nc.gpsimd.memset(vpt[:, :, DK : DK + 1], 1.0)
                        nc.gpsimd.tensor_copy(vpt[:, :, 0:DK], tmp[:])
                        vp_tiles[(h, b)] = vpt

                halves = []
                for b in range(BATCH):
                    for TT, w_sb, x_dram, t in (
                        (KT, wk_sb, kT, "k"),
                        (QT, wq_sb, qT, "q"),
                        (VT, wv_sb, vT, "v"),
                    ):
                        halves.append((TT, w_sb, x_dram, t, b))
                # reorder: K_b0, Q_b0 first (pre-emitted), then the rest
                halves = [halves[0], halves[1], halves[2]] + halves[3:]
                steps = []
                for TT, w_sb, x_dram, t, b in halves:
                    box = [None]
                    for k in range(KT_TILES):
                        steps.append(
                            lambda TT=TT, w_sb=w_sb, x_dram=x_dram, b=b, k=k,
                            box=box: emit_half_kstep(TT, w_sb, x_dram, b, k, box)
                        )
                    if t == "v":
                        steps.append(lambda b=b: emit_vplus(b))

                # pre-emit K_b0 + Q_b0 (16 k-steps)
                for s in steps[:16]:
                    s()
                rest = steps[16:]
                ri = 0

                def run_steps(n):
                    nonlocal ri
                    for _ in range(n):
                        if ri < len(rest):
                            rest[ri]()
                            ri += 1

                st0 = {"w": windows[0], "e": {}}
                for p in range(JC):
                    emit_scores(st0, p)
                    run_steps(2)

                st1 = {"w": windows[1], "e": {}}
                for p in range(8):
                    emit_scores(st1, p)
                    run_steps(1)
                run_steps(len(rest))

            with tc.tile_pool(name="pso", bufs=2, space="PSUM") as pso:
                for p in range(8, JC):
                    emit_av(st0, pso, AV1_CHUNKS[p])
                    emit_scores(st1, p)

                # steady windows: av -> wo -> scores per position, so the
                # score-psum slots have drained (exp) and the interleaved
                # h0/h1 score pairs schedule back-to-back into disjoint PE
                # row groups (concurrent streaming).
                st2 = {"w": windows[2], "e": {}}
                st3 = {"w": windows[3], "e": {}}
                for sc_st, av_st, wo_st in ((st2, st1, st0), (st3, st2, st1)):
                    emit_normalize(wo_st)
                    for p in range(JC):
                        emit_av(av_st, pso, AV_CHUNKS[p])
                        if p in WO_POS:
                            emit_wo(wo_st, WO_POS[p])
                        emit_scores(sc_st, p)

                # tail: dense av3 (2 chunks/pos) + wo2, then wo3 2-up
                emit_normalize(st2)
                for p in range(8):
                    emit_av(st3, pso, [2 * p, 2 * p + 1])
                    emit_wo(st2, p)
                emit_normalize(st3)
                for p in range(4):
                    emit_wo(st3, 2 * p)
                    emit_wo(st3, 2 * p + 1)

    nc.compile()
    return nc


def _prep_w(W, cs):
    # [HG, D] slice -> [128 p, KT_TILES ko, HG n] contiguous fp16
    a = W[cs, :].T.astype(np.float16)  # [D, HG]
    return np.ascontiguousarray(
        a.reshape(KT_TILES, 128, HG).transpose(1, 0, 2)
    )


def make_in_maps(q, k, v, Wq, Wk, Wv, Wo):
    qT = np.ascontiguousarray(q.reshape(M, D).T.astype(np.float16))
    kT = np.ascontiguousarray(k.reshape(M, D).T.astype(np.float16))
    vT = np.ascontiguousarray(v.reshape(M, D).T.astype(np.float16))
    in_maps = []
    for c in range(N_CORES):
        cs = slice(c * HG, (c + 1) * HG)
        in_maps.append(
            {
                "qT": qT,
                "kT": kT,
                "vT": vT,
                "wqT": _prep_w(Wq, cs),
                "wkT": _prep_w(Wk, cs),
                "wvT": _prep_w(Wv, cs),
                "woT": np.ascontiguousarray(Wo[:, cs].T),
            }
        )
    return in_maps


def kernel(q, k, v, Wq, Wk, Wv, Wo):
    q = np.asarray(q, dtype=np.float32)
    k = np.asarray(k, dtype=np.float32)
    v = np.asarray(v, dtype=np.float32)
    Wq = np.asarray(Wq, dtype=np.float32)
    Wk = np.asarray(Wk, dtype=np.float32)
    Wv = np.asarray(Wv, dtype=np.float32)
    Wo = np.asarray(Wo, dtype=np.float32)

    in_maps = make_in_maps(q, k, v, Wq, Wk, Wv, Wo)

    nc = build_bass()

    def run_once():
        res = run_bass_kernel_spmd(nc, in_maps, core_ids=list(range(N_CORES)))
        acc = res.results[0]["out"].astype(np.float32)
        for c in range(1, N_CORES):
            acc = acc + res.results[c]["out"].astype(np.float32)
        return acc

    acc = run_once()
    if not np.isfinite(acc).all():
        acc = run_once()  # guard against sporadic device flake
    return acc.reshape(BATCH, SEQ, D)

